# revision 2
# baseline (speedup 1.0000x reference)
# Causal self-attention (B=4, T=2048, C=1024, 16 heads) on 8 NeuronCores.
#
# Sharding: core i = (batch b = i//2, head-group g = i%2).  Each core runs the
# full attention pipeline for one batch element and 8 of the 16 heads:
#   qT,kT = Wqk^T @ x^T        (cols-on-partitions, bias on DVE eviction)
#   v     = x @ Wv + bv        (tokens-on-partitions; ones col appended/head)
#   S^T   = kT-tile^T @ qT     (keys on partitions, causal column trim)
#   P     = exp(S/8) * tri-mask on the 128-wide diagonal block only
#   y_aug = P-tile^T @ v_aug   (P STATIONARY, v moving: out [128q, 65] costs
#                               65 PE rows/tile instead of 512; col 64 = denom)
#   y     = y_aug[:, :64] * recip(y_aug[:, 64])    (per-partition broadcast)
#   yT    = XBAR dma transpose (q-part -> d-part)
#   out  += yT^T @ Wproj_pair  (partial over head-group; summed on host)
# Loops run query-chunk-outer / head-pair-inner so exp starts early and the
# output projection for chunk j overlaps chunk j+1's attention.
import numpy as np
import ml_dtypes

import concourse.tile as tile
from concourse import bacc, mybir
from concourse.bass_utils import run_bass_kernel_spmd

BF16 = mybir.dt.bfloat16
F32 = mybir.dt.float32
AF = mybir.ActivationFunctionType
MULT = mybir.AluOpType.mult
ADD = mybir.AluOpType.add

# Full-problem constants (hardcoded; kernel.py must be self-contained).
B, T, C, N_HEAD = 4, 2048, 1024, 16
CHECKPOINTS = []
D = C // N_HEAD            # 64
H = N_HEAD // 2            # 8 heads per core
GC = H * D                 # 512 group cols
P = 128


def build_nc(trace=False):
    """Build the single-core Bass/Tile program (shared SPMD across 8 cores)."""
    KC = C // P                 # 8 contraction chunks for C
    NT = T // P                 # 16 token tiles
    TQ = 512                    # query-chunk width
    NJ = T // TQ                # 4 query chunks
    NM = TQ // P                # 4 query subtiles per chunk
    GKC = GC // P               # 4 head pairs
    VW = D + 1                  # per-head v width incl. ones column

    nc = bacc.Bacc("TRN2", target_bir_lowering=False, debug=False)

    xT_d = nc.dram_tensor("xT", [C, T], BF16, kind="ExternalInput")
    # host layout: [C, pair, {q,k}, 128]
    wqk_d = nc.dram_tensor("wqk", [C, 2 * GC], BF16, kind="ExternalInput")
    bqk_d = nc.dram_tensor("bqk", [2 * GC], F32, kind="ExternalInput")
    wv_d = nc.dram_tensor("wv", [C, GC], BF16, kind="ExternalInput")
    bv_d = nc.dram_tensor("bv", [GC], F32, kind="ExternalInput")
    wp_d = nc.dram_tensor("wp", [GC, C], BF16, kind="ExternalInput")
    mask_d = nc.dram_tensor("mask", [P, P], BF16, kind="ExternalInput")
    out_d = nc.dram_tensor("out", [T, C], BF16, kind="ExternalOutput")

    with tile.TileContext(nc) as tc:
        with (
            tc.tile_pool(name="persist", bufs=1) as persist,
            tc.tile_pool(name="ptile", bufs=6) as ptile,
            tc.tile_pool(name="evict", bufs=4) as evict,
            tc.tile_pool(name="ynorm", bufs=3) as ynorm,
            tc.tile_pool(name="mm_psum", bufs=2, space="PSUM") as mm_psum,
            tc.tile_pool(name="s_psum", bufs=2, space="PSUM") as s_psum,
            tc.tile_pool(name="y_psum", bufs=1, space="PSUM") as y_psum,
        ):
            # ---- persistent SBUF tensors.  DMA issue order == first-use
            # order: wqk pair 0, xT chunk 0, wv (for v tiles / AV), masks+
            # biases, then the later xT chunks / wqk pairs / wp.
            wqk_sb = persist.tile([P, KC, GKC, 2, P], BF16)
            wqk_r = wqk_d.ap().rearrange(
                "(kc p) (pr two m) -> p kc pr two m", p=P, pr=GKC, two=2)
            xT_sb = persist.tile([P, KC, T], BF16)
            xT_r = xT_d.ap().rearrange("(kc p) t -> p kc t", p=P)
            wv_sb = persist.tile([P, KC, GC], BF16)
            wv_r = wv_d.ap().rearrange("(kc p) m -> p kc m", p=P)
            wp_sb = persist.tile([P, GKC, C], BF16)

            nc.sync.dma_start(wqk_sb[:, :, 0], wqk_r[:, :, 0])
            for q in range(4):
                nc.sync.dma_start(xT_sb[:, 2 * q:2 * q + 2, :TQ],
                                  xT_r[:, 2 * q:2 * q + 2, :TQ])
            nc.sync.dma_start(wv_sb[:, :KC // 2], wv_r[:, :KC // 2])
            nc.sync.dma_start(wv_sb[:, KC // 2:], wv_r[:, KC // 2:])
            mask_sb = persist.tile([P, P], BF16)
            nc.sync.dma_start(mask_sb[:], mask_d.ap())
            bqk_sb = persist.tile([P, 2 * GKC], F32)
            nc.sync.dma_start(bqk_sb[:], bqk_d.ap().rearrange("(kc p) -> p kc", p=P))
            bv_sb = persist.tile([1, GC], F32)
            nc.sync.dma_start(bv_sb[:], bv_d.ap()[None, :])
            bvb = persist.tile([P, GC], F32)
            nc.gpsimd.partition_broadcast(bvb[:], bv_sb[:])
            for j in range(1, NJ):
                js = slice(j * TQ, (j + 1) * TQ)
                nc.sync.dma_start(xT_sb[:, :, js], xT_r[:, :, js])
            for c in range(1, GKC):
                nc.sync.dma_start(wqk_sb[:, :, c], wqk_r[:, :, c])
            nc.sync.dma_start(wp_sb[:], wp_d.ap().rearrange("(kc p) m -> p kc m", p=P))

            # DVE "touch": absorb DMA waits into the DVE vector clock before
            # their first 2-input consumers.
            scrap = persist.tile([P, 2], F32)
            nc.vector.tensor_copy(scrap[:, 0:1], bqk_sb[:, 0:1])
            nc.vector.tensor_copy(scrap[:, 1:2], mask_sb[:, 0:1])

            qkT_sb = persist.tile([P, GKC, 2, T], BF16)
            v_sb = persist.tile([P, NT, H, VW], BF16)
            nc.vector.memset(v_sb[:, :, :, D:VW], 1.0)  # ones col per head
            yT_sb = persist.tile([P, GKC, T], BF16)

            # ---- v = x @ Wv (tokens on partitions), bias added on eviction.
            def v_tiles(trange):
                for t in trange:
                    ps = mm_psum.tile([P, GC], F32, tag="mm")
                    for kc in range(KC):
                        nc.tensor.matmul(
                            ps[:],
                            xT_sb[:, kc, t * P:(t + 1) * P],
                            wv_sb[:, kc, :],
                            start=(kc == 0), stop=(kc == KC - 1),
                        )
                    nc.vector.tensor_tensor(
                        v_sb[:, t, :, :D],
                        ps[:].rearrange("p (h e) -> p h e", h=H),
                        bvb[:].rearrange("p (h e) -> p h e", h=H),
                        ADD,
                    )

            # ---- qT,kT for (pair c, chunk j): qkT = Wqk^T @ x^T, bias on
            # DVE eviction.
            def qkT_group(c, j):
                for two in range(2):
                    ps = mm_psum.tile([P, TQ], F32, tag="mm")
                    for kc in range(KC):
                        nc.tensor.matmul(
                            ps[:],
                            wqk_sb[:, kc, c, two, :],
                            xT_sb[:, kc, j * TQ:(j + 1) * TQ],
                            start=(kc == 0), stop=(kc == KC - 1),
                        )
                    nc.vector.tensor_tensor(
                        qkT_sb[:, c, two, j * TQ:(j + 1) * TQ], ps[:],
                        bqk_sb[:, 2 * c + two:2 * c + two + 1].to_broadcast((P, TQ)),
                        ADD,
                    )

            scale = float(1.0 / np.sqrt(D))

            # ---- out_partial tile t: yT^T @ Wproj (all pairs)
            def proj_tile(t):
                ot = evict.tile([P, C], BF16, tag="out")
                for nn in range(C // TQ):
                    ps = mm_psum.tile([P, TQ], F32, tag="mm")
                    for kc in range(GKC):
                        nc.tensor.matmul(
                            ps[:],
                            yT_sb[:, kc, t * P:(t + 1) * P],
                            wp_sb[:, kc, nn * TQ:(nn + 1) * TQ],
                            start=(kc == 0), stop=(kc == GKC - 1),
                        )
                    nc.vector.tensor_copy(ot[:, nn * TQ:(nn + 1) * TQ], ps[:])
                nc.sync.dma_start(out_d.ap()[t * P:(t + 1) * P, :], ot[:])

            # ---- attention for (pair c, chunk j).  Scores S^T per key tile
            # (2 heads on disjoint PE row groups), exp from PSUM, triangular
            # mask on the diagonal 128-block only, then the flipped AV:
            # stationary = P block [128k, 128q], moving = v_aug [128k, 65]
            # (col 64 = softmax denominator).  Query tile m is normalized +
            # XBAR-transposed right after its diagonal key tile; `fillers`
            # are independent PE work units popped between key tiles so the
            # PE stays fed while the ACT exp stream is the local bottleneck.
            def attention(c, j, fillers):
                hA, hB = 2 * c, 2 * c + 1
                y_ps = y_psum.tile([P, NM, 2, P], F32, tag="y")

                def yslot(qtl):
                    return y_ps[:, qtl]

                # PSUM start=True clears the WHOLE bank's has_written bits,
                # so interleaved accumulation groups in one bank must not
                # each use start.  Clear each of the tile's 2 banks once via
                # a dummy 1-wide matmul into an unused column, then run every
                # AV matmul with start=False: the first write to an element
                # finds its bit clear and overwrites, later writes accumulate.
                for bank in (0, 2):
                    nc.tensor.matmul(y_ps[:, bank, 0, P - 1:P],
                                     mask_sb[:], mask_sb[:, 0:1],
                                     start=True, stop=True,
                                     skip_group_check=True)
                ilast = (j + 1) * NM - 1
                for i in range(ilast + 1):
                    m = i - j * NM
                    lo = P * m if m > 0 else 0
                    cs = slice(j * TQ + lo, (j + 1) * TQ)
                    ls = slice(lo, TQ)
                    st = s_psum.tile([P, 2, TQ], F32, tag="s")
                    kt = slice(i * P, (i + 1) * P)
                    nc.tensor.matmul(st[:, 0, ls], qkT_sb[:D, c, 1, kt],
                                     qkT_sb[:D, c, 0, cs], start=True, stop=True)
                    nc.tensor.matmul(st[:, 1, ls], qkT_sb[D:, c, 1, kt],
                                     qkT_sb[D:, c, 0, cs], start=True, stop=True)
                    pt = ptile.tile([P, 2, TQ], BF16, tag="p")
                    nc.scalar.activation(pt[:, :, ls], st[:, :, ls],
                                         AF.Exp, scale=scale)
                    if m >= 0:
                        ds = slice(lo, lo + P)
                        with tc.high_priority():
                            nc.vector.tensor_tensor(
                                pt[:, :, ds], pt[:, :, ds],
                                mask_sb[:, None, :].to_broadcast((P, 2, P)),
                                MULT)
                    for qtl in range(m if m > 0 else 0, NM):
                        qs = slice(qtl * P, (qtl + 1) * P)
                        last = (i == j * NM + qtl)
                        ys = yslot(qtl)
                        nc.tensor.matmul(
                            ys[:, 0, :VW],
                            pt[:, 0, qs], v_sb[:, i, hA, :],
                            start=False, stop=last, skip_group_check=True)
                        nc.tensor.matmul(
                            ys[:, 1, :VW],
                            pt[:, 1, qs], v_sb[:, i, hB, :],
                            start=False, stop=last, skip_group_check=True)
                    if m >= 0:
                        # query tile m complete: normalize + transpose now
                        qt = j * NM + m
                        ys = yslot(m)
                        rc = evict.tile([P, 2], F32, tag="rc")
                        nc.vector.reciprocal_approx_fast(rc[:], ys[:, :, D])
                        yn = ynorm.tile([P, 2, D], BF16, tag="yn")
                        nc.vector.tensor_tensor(
                            yn[:], ys[:, :, :D],
                            rc[:, :, None].to_broadcast((P, 2, D)), MULT)
                        nc.sync.dma_start_transpose(
                            yT_sb[:, c, qt * P:(qt + 1) * P],
                            yn[:].rearrange("p h e -> p (h e)"))
                    if fillers:
                        fillers.pop(0)()
                while fillers:
                    fillers.pop(0)()

            # ---- main schedule: chunk-outer, pair-inner; see `attention`.
            def ck(label):
                CHECKPOINTS.append((label, nc.next_id()))

            for j in range(NJ):
                for c in range(GKC):
                    fillers = []
                    # qkT for the pair whose attention comes next
                    if c + 1 < GKC:
                        nxt = (c + 1, j)
                    elif j + 1 < NJ:
                        nxt = (0, j + 1)
                    else:
                        nxt = None
                    if nxt is not None:
                        fillers.append(lambda cc=nxt[0], jj=nxt[1]:
                                       qkT_group(cc, jj))
                    # v tiles for the next chunk (one per pair)
                    if j + 1 < NJ:
                        fillers.append(lambda t=NM * (j + 1) + c: v_tiles([t]))
                    # output projections ride with the two LAST pairs of the
                    # next chunk (whose attention windows are ACT-starved);
                    # the final chunk's own projections trail as fillers of
                    # att(c3, j3), each ready ~2 tiles after its transpose.
                    if j > 0 and c >= 2:
                        for t in (NM * (j - 1) + 2 * (c - 2),
                                  NM * (j - 1) + 2 * (c - 2) + 1):
                            fillers.append(lambda t=t: proj_tile(t))
                    if j == 0 and c == 0:
                        ck("qkT(c0,j0)")
                        qkT_group(0, 0)
                        v_tiles(range(NM))
                    ck(f"att(c{c},j{j})")
                    attention(c, j, fillers)
            # final chunk's projections: their yT transposes were emitted
            # inside att(c3, j3); the scheduler overlaps what it can
            ck("tailproj")
            for t in range(NM * (NJ - 1), NM * NJ):
                proj_tile(t)
            ck("end")

    nc.compile()
    return nc


def make_mask():
    f = np.arange(P)[None, :]
    p = np.arange(P)[:, None]
    return (f >= p).astype(ml_dtypes.bfloat16)


def make_in_maps(x, W_attn, b_attn, W_proj):
    bf16 = ml_dtypes.bfloat16
    GKC_ = GC // P
    mask = make_mask()
    xTs = [np.ascontiguousarray(np.asarray(x[b]).T).astype(bf16)
           for b in range(B)]
    per_g = []
    for g in range(2):
        s = slice(g * GC, (g + 1) * GC)
        wq = W_attn[:, :C][:, s]          # [C, GC]
        wk = W_attn[:, C:2 * C][:, s]
        # [C, pair, {q,k}, 128]
        wqk = np.stack([wq.reshape(C, GKC_, P),
                        wk.reshape(C, GKC_, P)], axis=2)  # [C, pr, 2, 128]
        bq = b_attn[:C][s].reshape(GKC_, P)
        bk = b_attn[C:2 * C][s].reshape(GKC_, P)
        # bqk dram layout [2*GC] consumed as rearrange("(kc p) -> p kc"):
        # element (p, idx) at dram[idx*P + p]; want idx = 2*c+two
        bqk_flat = np.empty(2 * GC, np.float32)
        for c in range(GKC_):
            bqk_flat[(2 * c) * P:(2 * c + 1) * P] = bq[c]
            bqk_flat[(2 * c + 1) * P:(2 * c + 2) * P] = bk[c]
        per_g.append({
            "wqk": np.ascontiguousarray(wqk.reshape(C, 2 * GC)).astype(bf16),
            "bqk": bqk_flat,
            "wv": np.ascontiguousarray(W_attn[:, 2 * C:][:, s]).astype(bf16),
            "bv": b_attn[2 * C:][s].astype(np.float32),
            "wp": np.ascontiguousarray(W_proj[s, :]).astype(bf16),
            "mask": mask,
        })
    return [{"xT": xTs[core // 2], **per_g[core % 2]} for core in range(8)]


_NC_CACHE = {}


def kernel(x, W_attn, b_attn, W_proj, b_proj):
    x = np.asarray(x)
    W_attn = np.asarray(W_attn)
    b_attn = np.asarray(b_attn)
    W_proj = np.asarray(W_proj)
    b_proj = np.asarray(b_proj)

    if "nc" not in _NC_CACHE:
        _NC_CACHE["nc"] = build_nc()
    nc = _NC_CACHE["nc"]
    in_maps = make_in_maps(x, W_attn, b_attn, W_proj)
    try:
        res = run_bass_kernel_spmd(nc, in_maps, list(range(8)), trace=False)
    except Exception:
        # transient NRT_EXEC_UNIT_UNRECOVERABLE device wedges have been
        # observed on this fleet; one retry usually clears them
        import time as _time
        _time.sleep(5)
        res = run_bass_kernel_spmd(nc, in_maps, list(range(8)), trace=False)
    out = np.empty((B, T, C), np.float32)
    for b in range(B):
        out[b] = res.results[2 * b]["out"].astype(np.float32) \
            + res.results[2 * b + 1]["out"].astype(np.float32) \
            + b_proj[None, :]
    return out


# revision 3
# speedup vs baseline: 1.0158x; 1.0158x over previous
# Causal self-attention (B=4, T=2048, C=1024, 16 heads) on 8 NeuronCores.
#
# Sharding: core i = (batch b = i//2, head-group g = i%2).  Each core runs the
# full attention pipeline for one batch element and 8 of the 16 heads:
#   qT,kT = Wqk^T @ x^T        (cols-on-partitions, bias on DVE eviction)
#   v     = x @ Wv + bv        (tokens-on-partitions; ones col appended/head)
#   S^T   = kT-tile^T @ qT     (keys on partitions, causal column trim)
#   P     = exp(S/8) * tri-mask on the 128-wide diagonal block only
#   y_aug = P-tile^T @ v_aug   (P STATIONARY, v moving: out [128q, 65] costs
#                               65 PE rows/tile instead of 512; col 64 = denom)
#   y     = y_aug[:, :64] * recip(y_aug[:, 64])    (per-partition broadcast)
#   yT    = XBAR dma transpose (q-part -> d-part)
#   out  += yT^T @ Wproj_pair  (partial over head-group; summed on host)
# Loops run query-chunk-outer / head-pair-inner so exp starts early and the
# output projection for chunk j overlaps chunk j+1's attention.
import numpy as np
import ml_dtypes

import concourse.tile as tile
from concourse import bacc, mybir
from concourse.bass_utils import run_bass_kernel_spmd

BF16 = mybir.dt.bfloat16
F32 = mybir.dt.float32
AF = mybir.ActivationFunctionType
MULT = mybir.AluOpType.mult
ADD = mybir.AluOpType.add

# Full-problem constants (hardcoded; kernel.py must be self-contained).
B, T, C, N_HEAD = 4, 2048, 1024, 16
CHECKPOINTS = []
D = C // N_HEAD            # 64
H = N_HEAD // 2            # 8 heads per core
GC = H * D                 # 512 group cols
P = 128


def build_nc(trace=False):
    """Build the single-core Bass/Tile program (shared SPMD across 8 cores)."""
    KC = C // P                 # 8 contraction chunks for C
    NT = T // P                 # 16 token tiles
    TQ = 512                    # query-chunk width
    NJ = T // TQ                # 4 query chunks
    NM = TQ // P                # 4 query subtiles per chunk
    GKC = GC // P               # 4 head pairs
    VW = D + 1                  # per-head v width incl. ones column

    nc = bacc.Bacc("TRN2", target_bir_lowering=False, debug=False)

    xT_d = nc.dram_tensor("xT", [C, T], BF16, kind="ExternalInput")
    # host layout: [C, pair, {q,k}, 128]
    wqk_d = nc.dram_tensor("wqk", [C, 2 * GC], BF16, kind="ExternalInput")
    bqk_d = nc.dram_tensor("bqk", [2 * GC], F32, kind="ExternalInput")
    wv_d = nc.dram_tensor("wv", [C, GC], BF16, kind="ExternalInput")
    bv_d = nc.dram_tensor("bv", [GC], F32, kind="ExternalInput")
    wp_d = nc.dram_tensor("wp", [GC, C], BF16, kind="ExternalInput")
    mask_d = nc.dram_tensor("mask", [P, P], BF16, kind="ExternalInput")
    out_d = nc.dram_tensor("out", [T, C], BF16, kind="ExternalOutput")

    with tile.TileContext(nc) as tc:
        with (
            tc.tile_pool(name="persist", bufs=1) as persist,
            tc.tile_pool(name="ptile", bufs=6) as ptile,
            tc.tile_pool(name="evict", bufs=4) as evict,
            tc.tile_pool(name="ynorm", bufs=3) as ynorm,
            tc.tile_pool(name="mm_psum", bufs=2, space="PSUM") as mm_psum,
            tc.tile_pool(name="s_psum", bufs=2, space="PSUM") as s_psum,
            tc.tile_pool(name="y_psum", bufs=1, space="PSUM") as y_psum,
        ):
            # ---- persistent SBUF tensors.  DMA issue order == first-use
            # order: wqk pair 0, xT chunk 0, wv (for v tiles / AV), masks+
            # biases, then the later xT chunks / wqk pairs / wp.
            wqk_sb = persist.tile([P, KC, GKC, 2, P], BF16)
            wqk_r = wqk_d.ap().rearrange(
                "(kc p) (pr two m) -> p kc pr two m", p=P, pr=GKC, two=2)
            xT_sb = persist.tile([P, KC, T], BF16)
            xT_r = xT_d.ap().rearrange("(kc p) t -> p kc t", p=P)
            wv_sb = persist.tile([P, KC, GC], BF16)
            wv_r = wv_d.ap().rearrange("(kc p) m -> p kc m", p=P)
            wp_sb = persist.tile([P, GKC, C], BF16)

            nc.sync.dma_start(wqk_sb[:, :, 0], wqk_r[:, :, 0])
            for q in range(4):
                nc.sync.dma_start(xT_sb[:, 2 * q:2 * q + 2, :TQ],
                                  xT_r[:, 2 * q:2 * q + 2, :TQ])
            nc.sync.dma_start(wv_sb[:, :KC // 2], wv_r[:, :KC // 2])
            nc.sync.dma_start(wv_sb[:, KC // 2:], wv_r[:, KC // 2:])
            mask_sb = persist.tile([P, P], BF16)
            nc.sync.dma_start(mask_sb[:], mask_d.ap())
            bqk_sb = persist.tile([P, 2 * GKC], F32)
            nc.sync.dma_start(bqk_sb[:], bqk_d.ap().rearrange("(kc p) -> p kc", p=P))
            bv_sb = persist.tile([1, GC], F32)
            nc.sync.dma_start(bv_sb[:], bv_d.ap()[None, :])
            bvb = persist.tile([P, GC], F32)
            nc.gpsimd.partition_broadcast(bvb[:], bv_sb[:])
            for j in range(1, NJ):
                js = slice(j * TQ, (j + 1) * TQ)
                nc.sync.dma_start(xT_sb[:, :, js], xT_r[:, :, js])
            for c in range(1, GKC):
                nc.sync.dma_start(wqk_sb[:, :, c], wqk_r[:, :, c])
            nc.sync.dma_start(wp_sb[:], wp_d.ap().rearrange("(kc p) m -> p kc m", p=P))

            # DVE "touch": absorb DMA waits into the DVE vector clock before
            # their first 2-input consumers.
            scrap = persist.tile([P, 2], F32)
            nc.vector.tensor_copy(scrap[:, 0:1], bqk_sb[:, 0:1])
            nc.vector.tensor_copy(scrap[:, 1:2], mask_sb[:, 0:1])

            qkT_sb = persist.tile([P, GKC, 2, T], BF16)
            v_sb = persist.tile([P, NT, H, VW], BF16)
            nc.vector.memset(v_sb[:, :, :, D:VW], 1.0)  # ones col per head
            yT_sb = persist.tile([P, GKC, T], BF16)

            # ---- v = x @ Wv (tokens on partitions), bias added on eviction.
            def v_tiles(trange):
                for t in trange:
                    ps = mm_psum.tile([P, GC], F32, tag="mm")
                    for kc in range(KC):
                        nc.tensor.matmul(
                            ps[:],
                            xT_sb[:, kc, t * P:(t + 1) * P],
                            wv_sb[:, kc, :],
                            start=(kc == 0), stop=(kc == KC - 1),
                        )
                    nc.vector.tensor_tensor(
                        v_sb[:, t, :, :D],
                        ps[:].rearrange("p (h e) -> p h e", h=H),
                        bvb[:].rearrange("p (h e) -> p h e", h=H),
                        ADD,
                    )

            # ---- qT,kT for (pair c, chunk j): qkT = Wqk^T @ x^T, bias on
            # DVE eviction.
            def qkT_group(c, j):
                for two in range(2):
                    ps = mm_psum.tile([P, TQ], F32, tag="mm")
                    for kc in range(KC):
                        nc.tensor.matmul(
                            ps[:],
                            wqk_sb[:, kc, c, two, :],
                            xT_sb[:, kc, j * TQ:(j + 1) * TQ],
                            start=(kc == 0), stop=(kc == KC - 1),
                        )
                    nc.vector.tensor_tensor(
                        qkT_sb[:, c, two, j * TQ:(j + 1) * TQ], ps[:],
                        bqk_sb[:, 2 * c + two:2 * c + two + 1].to_broadcast((P, TQ)),
                        ADD,
                    )

            scale = float(1.0 / np.sqrt(D))

            # ---- out_partial tile t: yT^T @ Wproj (all pairs)
            def proj_tile(t):
                ot = evict.tile([P, C], BF16, tag="out")
                for nn in range(C // TQ):
                    ps = mm_psum.tile([P, TQ], F32, tag="mm")
                    for kc in range(GKC):
                        nc.tensor.matmul(
                            ps[:],
                            yT_sb[:, kc, t * P:(t + 1) * P],
                            wp_sb[:, kc, nn * TQ:(nn + 1) * TQ],
                            start=(kc == 0), stop=(kc == GKC - 1),
                        )
                    nc.vector.tensor_copy(ot[:, nn * TQ:(nn + 1) * TQ], ps[:])
                nc.sync.dma_start(out_d.ap()[t * P:(t + 1) * P, :], ot[:])

            # ---- attention for (pair c, chunk j).  Scores S^T per key tile
            # (2 heads on disjoint PE row groups), exp from PSUM, triangular
            # mask on the diagonal 128-block only, then the flipped AV:
            # stationary = P block [128k, 128q], moving = v_aug [128k, 65]
            # (col 64 = softmax denominator).  Query tile m is normalized +
            # XBAR-transposed right after its diagonal key tile; `fillers`
            # are independent PE work units popped between key tiles so the
            # PE stays fed while the ACT exp stream is the local bottleneck.
            def attention(c, j, fillers):
                hA, hB = 2 * c, 2 * c + 1
                y_ps = y_psum.tile([P, NM, 2, P], F32, tag="y")

                def yslot(qtl):
                    return y_ps[:, qtl]

                # PSUM start=True clears the WHOLE bank's has_written bits,
                # so interleaved accumulation groups in one bank must not
                # each use start.  Clear each of the tile's 2 banks once via
                # a dummy 1-wide matmul into an unused column, then run every
                # AV matmul with start=False: the first write to an element
                # finds its bit clear and overwrites, later writes accumulate.
                for bank in (0, 2):
                    nc.tensor.matmul(y_ps[:, bank, 0, P - 1:P],
                                     mask_sb[:], mask_sb[:, 0:1],
                                     start=True, stop=True,
                                     skip_group_check=True)
                ilast = (j + 1) * NM - 1
                for i in range(ilast + 1):
                    m = i - j * NM
                    lo = P * m if m > 0 else 0
                    cs = slice(j * TQ + lo, (j + 1) * TQ)
                    ls = slice(lo, TQ)
                    st = s_psum.tile([P, 2, TQ], F32, tag="s")
                    kt = slice(i * P, (i + 1) * P)
                    nc.tensor.matmul(st[:, 0, ls], qkT_sb[:D, c, 1, kt],
                                     qkT_sb[:D, c, 0, cs], start=True, stop=True)
                    nc.tensor.matmul(st[:, 1, ls], qkT_sb[D:, c, 1, kt],
                                     qkT_sb[D:, c, 0, cs], start=True, stop=True)
                    pt = ptile.tile([P, 2, TQ], BF16, tag="p")
                    nc.scalar.activation(pt[:, :, ls], st[:, :, ls],
                                         AF.Exp, scale=scale)
                    if m >= 0:
                        ds = slice(lo, lo + P)
                        with tc.high_priority():
                            nc.vector.tensor_tensor(
                                pt[:, :, ds], pt[:, :, ds],
                                mask_sb[:, None, :].to_broadcast((P, 2, P)),
                                MULT)
                    for qtl in range(m if m > 0 else 0, NM):
                        qs = slice(qtl * P, (qtl + 1) * P)
                        last = (i == j * NM + qtl)
                        ys = yslot(qtl)
                        nc.tensor.matmul(
                            ys[:, 0, :VW],
                            pt[:, 0, qs], v_sb[:, i, hA, :],
                            start=False, stop=last, skip_group_check=True)
                        nc.tensor.matmul(
                            ys[:, 1, :VW],
                            pt[:, 1, qs], v_sb[:, i, hB, :],
                            start=False, stop=last, skip_group_check=True)
                    if m >= 0:
                        # query tile m complete: normalize + transpose now
                        qt = j * NM + m
                        ys = yslot(m)
                        rc = evict.tile([P, 2], F32, tag="rc")
                        nc.vector.reciprocal_approx_fast(rc[:], ys[:, :, D])
                        yn = ynorm.tile([P, 2, D], BF16, tag="yn")
                        nc.vector.tensor_tensor(
                            yn[:], ys[:, :, :D],
                            rc[:, :, None].to_broadcast((P, 2, D)), MULT)
                        nc.sync.dma_start_transpose(
                            yT_sb[:, c, qt * P:(qt + 1) * P],
                            yn[:].rearrange("p h e -> p (h e)"))
                    while fillers and fillers[0][0] <= i:
                        fillers.pop(0)[1]()
                while fillers:
                    fillers.pop(0)[1]()

            # ---- main schedule: chunk-outer, pair-inner; see `attention`.
            def ck(label):
                CHECKPOINTS.append((label, nc.next_id()))

            for j in range(NJ):
                for c in range(GKC):
                    fillers = []
                    # qkT for the pair whose attention comes next
                    if c + 1 < GKC:
                        nxt = (c + 1, j)
                    elif j + 1 < NJ:
                        nxt = (0, j + 1)
                    else:
                        nxt = None
                    if nxt is not None:
                        fillers.append(lambda cc=nxt[0], jj=nxt[1]:
                                       qkT_group(cc, jj))
                    # v tiles for the next chunk (one per pair)
                    if j + 1 < NJ:
                        fillers.append(lambda t=NM * (j + 1) + c: v_tiles([t]))
                    # output projections ride with the two LAST pairs of the
                    # next chunk (whose attention windows are ACT-starved);
                    # the final chunk's own projections trail as fillers of
                    # att(c3, j3), each ready ~2 tiles after its transpose.
                    if j > 0 and c >= 2:
                        for t in (NM * (j - 1) + 2 * (c - 2),
                                  NM * (j - 1) + 2 * (c - 2) + 1):
                            fillers.append(lambda t=t: proj_tile(t))
                    if j == 0 and c == 0:
                        ck("qkT(c0,j0)")
                        qkT_group(0, 0)
                        v_tiles(range(NM))
                    # pace filler pops across the key-tile loop: first unit
                    # (qkT for the next attention) early, the rest spread so
                    # the late, ACT-starved tiles keep the PE fed
                    ntile = (j + 1) * NM
                    n = len(fillers)
                    if n <= 1:
                        pos = [ntile // 2]
                    else:
                        pos = [0] + [max(1, round(k * (ntile - 1) / (n - 1)))
                                     for k in range(1, n)]
                    fillers = [[p, f] for p, f in zip(pos, fillers)]
                    ck(f"att(c{c},j{j})")
                    attention(c, j, fillers)
            # final chunk's projections: their yT transposes were emitted
            # inside att(c3, j3); the scheduler overlaps what it can
            ck("tailproj")
            for t in range(NM * (NJ - 1), NM * NJ):
                proj_tile(t)
            ck("end")

    nc.compile()
    return nc


def make_mask():
    f = np.arange(P)[None, :]
    p = np.arange(P)[:, None]
    return (f >= p).astype(ml_dtypes.bfloat16)


def make_in_maps(x, W_attn, b_attn, W_proj):
    bf16 = ml_dtypes.bfloat16
    GKC_ = GC // P
    mask = make_mask()
    xTs = [np.ascontiguousarray(np.asarray(x[b]).T).astype(bf16)
           for b in range(B)]
    per_g = []
    for g in range(2):
        s = slice(g * GC, (g + 1) * GC)
        wq = W_attn[:, :C][:, s]          # [C, GC]
        wk = W_attn[:, C:2 * C][:, s]
        # [C, pair, {q,k}, 128]
        wqk = np.stack([wq.reshape(C, GKC_, P),
                        wk.reshape(C, GKC_, P)], axis=2)  # [C, pr, 2, 128]
        bq = b_attn[:C][s].reshape(GKC_, P)
        bk = b_attn[C:2 * C][s].reshape(GKC_, P)
        # bqk dram layout [2*GC] consumed as rearrange("(kc p) -> p kc"):
        # element (p, idx) at dram[idx*P + p]; want idx = 2*c+two
        bqk_flat = np.empty(2 * GC, np.float32)
        for c in range(GKC_):
            bqk_flat[(2 * c) * P:(2 * c + 1) * P] = bq[c]
            bqk_flat[(2 * c + 1) * P:(2 * c + 2) * P] = bk[c]
        per_g.append({
            "wqk": np.ascontiguousarray(wqk.reshape(C, 2 * GC)).astype(bf16),
            "bqk": bqk_flat,
            "wv": np.ascontiguousarray(W_attn[:, 2 * C:][:, s]).astype(bf16),
            "bv": b_attn[2 * C:][s].astype(np.float32),
            "wp": np.ascontiguousarray(W_proj[s, :]).astype(bf16),
            "mask": mask,
        })
    return [{"xT": xTs[core // 2], **per_g[core % 2]} for core in range(8)]


_NC_CACHE = {}


def kernel(x, W_attn, b_attn, W_proj, b_proj):
    x = np.asarray(x)
    W_attn = np.asarray(W_attn)
    b_attn = np.asarray(b_attn)
    W_proj = np.asarray(W_proj)
    b_proj = np.asarray(b_proj)

    if "nc" not in _NC_CACHE:
        _NC_CACHE["nc"] = build_nc()
    nc = _NC_CACHE["nc"]
    in_maps = make_in_maps(x, W_attn, b_attn, W_proj)
    try:
        res = run_bass_kernel_spmd(nc, in_maps, list(range(8)), trace=False)
    except Exception:
        # transient NRT_EXEC_UNIT_UNRECOVERABLE device wedges have been
        # observed on this fleet; one retry usually clears them
        import time as _time
        _time.sleep(5)
        res = run_bass_kernel_spmd(nc, in_maps, list(range(8)), trace=False)
    out = np.empty((B, T, C), np.float32)
    for b in range(B):
        out[b] = res.results[2 * b]["out"].astype(np.float32) \
            + res.results[2 * b + 1]["out"].astype(np.float32) \
            + b_proj[None, :]
    return out
